# revision 9
# baseline (speedup 1.0000x reference)
"""Trainium2 Bass kernel for DescartesExtension (order-2 polynomial feature map).

reference: out[b, n(i,j)] = x[b,i] * x[b,j] for i<=j in row-major upper-tri order,
x: [256, 1024] f32 -> out: [256, 524800] f32.

Structure used: for fixed i, output columns [off(i), off(i)+D-i) are
x[b,i] * x[b, i:D] -- a per-partition scalar times a contiguous slice
(tensor_scalar_mul on the DVE, batch rows on partitions).

Sharding (SPMD: one program, 8 cores, per-core differences only in input data):
core c handles segments i = c + 8k, k = 0..127.  Slot k runs a UNIFORM-width op
T_k = 1024 - 8k on a host-shifted input xs_c[b, t] = x[b, t+c] (zero padded), so
every AP in the program is identical across cores.  Core c's slot k therefore
computes its segment (length T_k - c) plus c trailing zeros.  Each core writes a
packed private output [256, 66048]; the host scatters slots back into the full
output and drops the padding tails.
"""

import numpy as np

B = 256
D = 1024
NCORES = 8
NSLOT = D // NCORES  # 128 slots per core
T = [D - NCORES * k for k in range(NSLOT)]  # uniform slot widths 1024, 1016, ..., 8
S = [0] * (NSLOT + 1)  # packed slot offsets
for _k in range(NSLOT):
    S[_k + 1] = S[_k] + T[_k]
OUTW = S[NSLOT]  # 66048 packed columns per core
CHUNK_MAX = 10240  # packed-output SBUF chunk width (40KB/partition f32)
BUFS = 4  # packed-chunk double buffering depth
RAMP = (1, 2, 4)  # slot counts of the pipeline-fill chunks in block 0

_prog_cache = None


def _chunks(ramp):
    """Group slots into chunks of <= CHUNK_MAX packed columns.

    `ramp` pre-slices a few tiny chunks at the front so the first store can
    issue almost immediately (pipeline fill), then greedy-packs the rest.
    """
    out = []
    k = 0
    for n in ramp:
        e = min(k + n, NSLOT)
        if e > k:
            out.append((k, e, S[k], S[e] - S[k]))
            k = e
    while k < NSLOT:
        e, w = k, 0
        while e < NSLOT and w + T[e] <= CHUNK_MAX:
            w += T[e]
            e += 1
        out.append((k, e, S[k], w))
        k = e
    return out


def _build_program():
    global _prog_cache
    if _prog_cache is not None:
        return _prog_cache

    import concourse.bacc as bacc
    import concourse.mybir as mybir
    import concourse.tile as tile

    nc = bacc.Bacc("TRN2", target_bir_lowering=False, debug=False)
    xs = nc.dram_tensor("xs", [B, D], mybir.dt.float32, kind="ExternalInput").ap()
    out = nc.dram_tensor("out", [B, OUTW], mybir.dt.float32, kind="ExternalOutput").ap()

    with tile.TileContext(nc) as tc:
        with (
            tc.tile_pool(name="xp", bufs=1) as xp,
            tc.tile_pool(name="op", bufs=BUFS) as op,
        ):
            si = 0
            for blk in range(B // 128):
                xb = xp.tile([128, D], mybir.dt.float32, tag=f"x{blk}")
                # scalar (ACT) is also a HWDGE issuer; loads there don't queue
                # behind the output stores on sync's ring.
                nc.scalar.dma_start(xb[:], xs[blk * 128 : (blk + 1) * 128, :])
                for k0, k1, c0, w in _chunks(RAMP if blk == 0 else ()):
                    pt = op.tile([128, CHUNK_MAX], mybir.dt.float32, tag="packed")
                    for k in range(k0, k1):
                        lo = S[k] - c0
                        nc.vector.tensor_scalar_mul(
                            out=pt[:, lo : lo + T[k]],
                            in0=xb[:, NCORES * k : NCORES * k + T[k]],
                            scalar1=xb[:, NCORES * k : NCORES * k + 1],
                        )
                    # alternate the two HWDGE issuing engines so descriptor
                    # generation/completion of consecutive stores overlap
                    eng = nc.sync if si % 2 == 0 else nc.scalar
                    si += 1
                    eng.dma_start(
                        out[blk * 128 : (blk + 1) * 128, c0 : c0 + w], pt[:, :w]
                    )
    nc.compile()
    _prog_cache = nc
    return nc


def _run(x, trace=False, trace_cores=None):
    """Returns (full_output, BassKernelResults)."""
    from concourse.bass_utils import run_bass_kernel_spmd

    x = np.ascontiguousarray(np.asarray(x), dtype=np.float32)
    assert x.shape == (B, D)
    nc = _build_program()

    in_maps = []
    for c in range(NCORES):
        xsc = np.zeros((B, D), np.float32)
        xsc[:, : D - c] = x[:, c:]
        in_maps.append({"xs": xsc})

    kw = {}
    if trace:
        kw["trace"] = True
        if trace_cores is not None:
            kw["trace_cores"] = trace_cores
    res = run_bass_kernel_spmd(nc, in_maps, core_ids=list(range(NCORES)), **kw)

    off = np.zeros(D + 1, np.int64)
    off[1:] = np.cumsum(D - np.arange(D))
    full = np.empty((B, D * (D + 1) // 2), np.float32)
    for c in range(NCORES):
        r = res.results[c]["out"]
        for k in range(NSLOT):
            i = c + NCORES * k
            L = D - i
            full[:, off[i] : off[i] + L] = r[:, S[k] : S[k] + L]
    return full, res


def kernel(x):
    return _run(x)[0]
